# revision 8
# baseline (speedup 1.0000x reference)
"""Trainium2 Bass kernel for a dense transformer block (B=8, S=2048, D=768, H=3072).

Sharding: pure data-parallel over batch -- one batch element per NeuronCore (8 cores).
All matmuls run as float32r (full PE rate at moving-dim >= 256, ~1.7e-4 rel err).

Layout strategy (per core, avoids all activation transposes except LN outputs):
  hT  [D, S]  feature-major   <- LN1 + PE transpose
  qT,kT [D,S] feature-major   <- lhsT=W, rhs=hT
  v   [S, D]  token-major     <- lhsT=hT, rhs=Wv
  scoresT [S2, S1-chunk]      <- lhsT=kT-slice, rhs=qT-chunk; exp fused on ACT
  yT  [D, S1] feature-major   <- lhsT=v-slice, rhs=expT; Z via ones-matmul
  o   [S1, D] token-major     <- lhsT=yT-slice, rhs=Wo; + residual -> x2
  h2T [D, S]  feature-major   <- LN2 + PE transpose
  uT/mT [H, S1] feature-major <- lhsT=Wfc-slice, rhs=h2T; GELU fused on ACT
  out [S1, D] token-major     <- lhsT=mT-slice, rhs=Wproj; + residual
"""

import numpy as np

P = 128
S, D, H = 2048, 768, 3072
DT = D // P            # 6 d-tiles
HT = H // P            # 24 h-tiles
ST = S // P            # 16 token tiles
CH = 512               # s1 chunk width
NCH = S // CH          # 4 chunks
TPC = CH // P          # 4 token tiles per chunk
D2C = 384              # d2 output chunk (psum bank limit 512 fp32; 2x384)
EPS = 1e-5
N_CORES = 8

WEIGHT_NAMES = [
    "ln1_g", "ln1_b", "ln2_g", "ln2_b",
    "Wq", "bq", "Wk", "bk", "Wv", "bv", "Wo", "bo",
    "Wfc", "bfc", "Wproj", "bproj",
]

_CACHE = {}


def _build():
    import concourse.bass as bass
    import concourse.tile as tile
    from concourse import bacc, mybir
    from concourse.masks import make_identity
    from contextlib import ExitStack

    F = mybir.dt.float32
    R = mybir.dt.float32r
    AF = mybir.ActivationFunctionType
    OP = mybir.AluOpType

    nc = bacc.Bacc(None, target_bir_lowering=False)

    x_d = nc.dram_tensor("x", [S, D], F, kind="ExternalInput")
    w_d = {}
    for nm in WEIGHT_NAMES:
        if nm.startswith("W"):
            shp = [D, H] if nm == "Wfc" else ([H, D] if nm == "Wproj" else [D, D])
        else:
            shp = [H] if nm == "bfc" else [D]
        w_d[nm] = nc.dram_tensor(nm, shp, F, kind="ExternalInput")
    out_d = nc.dram_tensor("out", [S, D], F, kind="ExternalOutput")

    def bcast_ap(dram_t, n_part=P):
        ap = dram_t.ap()
        return bass.AP(tensor=ap.tensor, offset=ap.offset, ap=[[0, n_part]] + list(ap.ap))

    inv_sqrt_d = 1.0 / float(np.sqrt(np.float32(D)))

    with tile.TileContext(nc) as tc, ExitStack() as ctx:
        singles = ctx.enter_context(tc.tile_pool(name="singles", bufs=1))
        dram = ctx.enter_context(tc.tile_pool(name="dram", bufs=1, space="DRAM"))

        # DRAM scratch
        q_scr = dram.tile([DT, P, S], R)       # qT spilled
        v_scr = dram.tile([ST, P, D], R)       # v token-major tiles
        x2_scr = dram.tile([ST, P, D], F)      # post-attention residual stream
        o2_scr = dram.tile([ST, P, D], F)      # MLP half-0 partial output

        # persistent constants
        ident = singles.tile([P, P], F)
        make_identity(nc, ident)
        ones_f = singles.tile([P, P], F)
        nc.vector.memset(ones_f, 1.0)
        ones_sb = singles.tile([P, P], R)
        nc.vector.tensor_copy(out=ones_sb, in_=ones_f)
        eps_t = singles.tile([P, 1], F)
        nc.vector.memset(eps_t, EPS)
        bo_bc = singles.tile([P, D], F)
        nc.gpsimd.dma_start(out=bo_bc, in_=bcast_ap(w_d["bo"]))
        bp_bc = singles.tile([P, D], F)
        nc.gpsimd.dma_start(out=bp_bc, in_=bcast_ap(w_d["bproj"]))
        bq_col = singles.tile([P, DT], F)
        nc.sync.dma_start(bq_col, w_d["bq"].ap().rearrange("(t p) -> p t", p=P))
        bk_col = singles.tile([P, DT], F)
        nc.sync.dma_start(bk_col, w_d["bk"].ap().rearrange("(t p) -> p t", p=P))
        bfc_col = singles.tile([P, HT], F)
        nc.sync.dma_start(bfc_col, w_d["bfc"].ap().rearrange("(t p) -> p t", p=P))

        kT_ctx = ExitStack()
        kT = kT_ctx.enter_context(tc.tile_pool(name="kT", bufs=1))
        kT_sb = kT.tile([P, DT, S], R)

        # ---------------- Phase 1: LN1 + transpose -> hT ----------------
        # ---------------- Phase 2: qT,kT,v ----------------
        with (
            tc.tile_pool(name="ph12", bufs=3) as ph12,
            tc.tile_pool(name="ln1c", bufs=1) as ln1c,
            tc.tile_pool(name="hT", bufs=1) as hTp,
            tc.tile_pool(name="wqkv", bufs=1) as wqkv,
            tc.tile_pool(name="ps12", bufs=2, space="PSUM") as ps12,
            tc.tile_pool(name="ps12b", bufs=2, space="PSUM") as ps12b,
        ):
            g1_bc = ln1c.tile([P, D], F)
            nc.gpsimd.dma_start(out=g1_bc, in_=bcast_ap(w_d["ln1_g"]))
            b1_bc = ln1c.tile([P, D], F)
            nc.gpsimd.dma_start(out=b1_bc, in_=bcast_ap(w_d["ln1_b"]))
            bv_bc = ln1c.tile([P, D], F)
            nc.gpsimd.dma_start(out=bv_bc, in_=bcast_ap(w_d["bv"]))

            hT_sb = hTp.tile([P, DT, S], R)
            for st in range(ST):
                x_t = ph12.tile([P, D], F, tag="xt")
                nc.sync.dma_start(x_t, x_d.ap()[st * P:(st + 1) * P, :])
                stats = ph12.tile([P, 3, 6], F, tag="st")
                for i in range(3):
                    nc.vector.bn_stats(out=stats[:, i, :], in_=x_t[:, i * 256:(i + 1) * 256])
                mv = ph12.tile([P, 2], F, tag="mv")
                nc.vector.bn_aggr(out=mv, in_=stats)
                rs = ph12.tile([P, 1], F, tag="rs")
                nc.scalar.activation(out=rs, in_=mv[:, 1:2], func=AF.Sqrt, bias=eps_t, scale=1.0)
                nc.vector.reciprocal(out=rs, in_=rs)
                h_t = ph12.tile([P, D], F, tag="ht")
                nc.vector.tensor_scalar(out=h_t, in0=x_t, scalar1=mv[:, 0:1], scalar2=rs,
                                        op0=OP.subtract, op1=OP.mult)
                nc.vector.tensor_tensor(out=h_t, in0=h_t, in1=g1_bc, op=OP.mult)
                nc.vector.tensor_tensor(out=h_t, in0=h_t, in1=b1_bc, op=OP.add)
                for dt_ in range(DT):
                    ps_tr = ps12.tile([P, P], F, tag="tr")
                    nc.tensor.transpose(ps_tr, h_t[:, dt_ * P:(dt_ + 1) * P], ident)
                    nc.vector.tensor_copy(out=hT_sb[:, dt_, st * P:(st + 1) * P], in_=ps_tr)

            # kT and qT: feature-major
            for nm, bcol, store_k in (("Wk", bk_col, True), ("Wq", bq_col, False)):
                w_t = wqkv.tile([P, DT, D], R, tag="w")
                for dt_ in range(DT):
                    nc.gpsimd.dma_start(w_t[:, dt_], w_d[nm].ap()[dt_ * P:(dt_ + 1) * P, :])
                for dtp in range(DT):
                    for sc in range(NCH):
                        ps = ps12b.tile([P, CH], F, tag="mm")
                        for dt_ in range(DT):
                            nc.tensor.matmul(
                                ps,
                                w_t[:, dt_, dtp * P:(dtp + 1) * P],
                                hT_sb[:, dt_, sc * CH:(sc + 1) * CH],
                                start=(dt_ == 0), stop=(dt_ == DT - 1))
                        if store_k:
                            nc.vector.tensor_scalar(out=kT_sb[:, dtp, sc * CH:(sc + 1) * CH],
                                                    in0=ps, scalar1=bcol[:, dtp:dtp + 1],
                                                    scalar2=None, op0=OP.add)
                        else:
                            q_sb = ph12.tile([P, CH], R, tag="qsb")
                            nc.vector.tensor_scalar(out=q_sb, in0=ps,
                                                    scalar1=bcol[:, dtp:dtp + 1],
                                                    scalar2=None, op0=OP.add)
                            nc.sync.dma_start(q_scr[dtp, :, sc * CH:(sc + 1) * CH], q_sb)

            # v: token-major
            wv_t = wqkv.tile([P, DT, D], R, tag="w")
            for dt_ in range(DT):
                nc.gpsimd.dma_start(wv_t[:, dt_], w_d["Wv"].ap()[dt_ * P:(dt_ + 1) * P, :])
            for st2 in range(ST):
                v_sb = ph12.tile([P, D], R, tag="vsb")
                for dc in range(2):
                    ps = ps12b.tile([P, D2C], F, tag="mmv")
                    for dt_ in range(DT):
                        nc.tensor.matmul(
                            ps,
                            hT_sb[:, dt_, st2 * P:(st2 + 1) * P],
                            wv_t[:, dt_, dc * D2C:(dc + 1) * D2C],
                            start=(dt_ == 0), stop=(dt_ == DT - 1))

                    nc.vector.tensor_tensor(out=v_sb[:, dc * D2C:(dc + 1) * D2C], in0=ps,
                                            in1=bv_bc[:, dc * D2C:(dc + 1) * D2C], op=OP.add)
                nc.sync.dma_start(v_scr[st2], v_sb)

        # ---------------- Phase 3: attention ----------------
        with (
            tc.tile_pool(name="ph3", bufs=3) as ph3,
            tc.tile_pool(name="exp", bufs=ST + 1) as expp,
            tc.tile_pool(name="yt", bufs=2) as ytp,
            tc.tile_pool(name="wo", bufs=1) as wop,
            tc.tile_pool(name="ps_a", bufs=1, space="PSUM") as ps_a,
            tc.tile_pool(name="ps_z", bufs=1, space="PSUM") as ps_z,
            tc.tile_pool(name="ps_y", bufs=6, space="PSUM") as ps_y,
        ):
            wo_t = wop.tile([P, DT, D], R)
            for dt_ in range(DT):
                nc.gpsimd.dma_start(wo_t[:, dt_], w_d["Wo"].ap()[dt_ * P:(dt_ + 1) * P, :])

            for sc in range(NCH):
                qTc = ph3.tile([P, DT, CH], R, tag="qtc")
                nc.sync.dma_start(
                    qTc, q_scr[:, :, sc * CH:(sc + 1) * CH].rearrange("t p n -> p t n"))

                # A: scoresT + exp
                exp_tiles = []
                for st2 in range(ST):
                    ps = ps_a.tile([P, CH], F, tag="sc")
                    for dt_ in range(DT):
                        nc.tensor.matmul(
                            ps,
                            kT_sb[:, dt_, st2 * P:(st2 + 1) * P],
                            qTc[:, dt_],
                            start=(dt_ == 0), stop=(dt_ == DT - 1))
                    e_t = expp.tile([P, CH], R, tag="exp")
                    nc.scalar.activation(out=e_t, in_=ps, func=AF.Exp, scale=inv_sqrt_d)
                    exp_tiles.append(e_t)

                # B: yT accumulation + Z
                ps_ys = [ps_y.tile([P, CH], F, tag="y", name=f"ps_y{i}")
                         for i in range(DT)]
                ps_zt = ps_z.tile([P, CH], F, tag="z", name="ps_zt")
                for st2 in range(ST):
                    v_t = ph3.tile([P, D], R, tag="vt")
                    nc.sync.dma_start(v_t, v_scr[st2])
                    e_r = exp_tiles[st2][:]
                    nc.tensor.matmul(ps_zt, ones_sb[:], e_r,
                                     start=(st2 == 0), stop=(st2 == ST - 1))
                    for dtp in range(DT):
                        nc.tensor.matmul(ps_ys[dtp],
                                         v_t[:, dtp * P:(dtp + 1) * P], e_r,
                                         start=(st2 == 0), stop=(st2 == ST - 1))
                rz = ph3.tile([P, CH], F, tag="rz")
                nc.vector.reciprocal(out=rz, in_=ps_zt)
                yT_sb = ytp.tile([P, DT, CH], R, tag="yt")
                for dtp in range(DT):
                    nc.vector.tensor_tensor(out=yT_sb[:, dtp], in0=ps_ys[dtp], in1=rz,
                                            op=OP.mult)

                # C: o = yT.T @ Wo, + x residual -> x2
                for su in range(TPC):
                    st = sc * TPC + su
                    x_t = ph3.tile([P, D], F, tag="xt3")
                    nc.sync.dma_start(x_t, x_d.ap()[st * P:(st + 1) * P, :])
                    for dc in range(2):
                        ps = ps_a.tile([P, D2C], F, tag="sc")
                        for dtp in range(DT):
                            nc.tensor.matmul(
                                ps,
                                yT_sb[:, dtp, su * P:(su + 1) * P],
                                wo_t[:, dtp, dc * D2C:(dc + 1) * D2C],
                                start=(dtp == 0), stop=(dtp == DT - 1))
                        sl = slice(dc * D2C, (dc + 1) * D2C)
                        nc.vector.tensor_tensor(out=x_t[:, sl], in0=x_t[:, sl], in1=ps, op=OP.add)
                    nc.vector.tensor_tensor(out=x_t, in0=x_t, in1=bo_bc, op=OP.add)
                    nc.sync.dma_start(x2_scr[st], x_t)

        kT_ctx.close()

        # ---------------- Phase 4: LN2 + transpose -> h2T scratch ----------------
        h2_scr = dram.tile([DT, P, S], R)
        with (
            tc.tile_pool(name="ph4", bufs=3) as ph4,
            tc.tile_pool(name="ln2c", bufs=1) as ln2c,
            tc.tile_pool(name="ps4", bufs=2, space="PSUM") as ps4,
        ):
            g2_bc = ln2c.tile([P, D], F)
            nc.gpsimd.dma_start(out=g2_bc, in_=bcast_ap(w_d["ln2_g"]))
            b2_bc = ln2c.tile([P, D], F)
            nc.gpsimd.dma_start(out=b2_bc, in_=bcast_ap(w_d["ln2_b"]))
            for st in range(ST):
                x2_t = ph4.tile([P, D], F, tag="x2")
                nc.sync.dma_start(x2_t, x2_scr[st])
                stats = ph4.tile([P, 3, 6], F, tag="st4")
                for i in range(3):
                    nc.vector.bn_stats(out=stats[:, i, :],
                                       in_=x2_t[:, i * 256:(i + 1) * 256])
                mv = ph4.tile([P, 2], F, tag="mv4")
                nc.vector.bn_aggr(out=mv, in_=stats)
                rs = ph4.tile([P, 1], F, tag="rs4")
                nc.scalar.activation(out=rs, in_=mv[:, 1:2], func=AF.Sqrt,
                                     bias=eps_t, scale=1.0)
                nc.vector.reciprocal(out=rs, in_=rs)
                h2_t = ph4.tile([P, D], F, tag="h2")
                nc.vector.tensor_scalar(out=h2_t, in0=x2_t, scalar1=mv[:, 0:1],
                                        scalar2=rs, op0=OP.subtract, op1=OP.mult)
                nc.vector.tensor_tensor(out=h2_t, in0=h2_t, in1=g2_bc, op=OP.mult)
                nc.vector.tensor_tensor(out=h2_t, in0=h2_t, in1=b2_bc, op=OP.add)
                h2T_t = ph4.tile([P, DT, P], R, tag="h2T")
                for dt_ in range(DT):
                    ps_tr = ps4.tile([P, P], F, tag="tr4")
                    nc.tensor.transpose(ps_tr, h2_t[:, dt_ * P:(dt_ + 1) * P], ident)
                    nc.vector.tensor_copy(out=h2T_t[:, dt_], in_=ps_tr)
                nc.sync.dma_start(
                    h2_scr[:, :, st * P:(st + 1) * P].rearrange("t p n -> p t n"), h2T_t)

        # ---------------- Phase 5: MLP (two H halves) ----------------
        HHT = HT // 2  # 12 h-tiles per half
        with (
            tc.tile_pool(name="ph5", bufs=2) as ph5,
            tc.tile_pool(name="mt", bufs=2) as mtp,
            tc.tile_pool(name="wmlp", bufs=1) as wmlp,
            tc.tile_pool(name="ps_u", bufs=2, space="PSUM") as ps_u,
            tc.tile_pool(name="ps_o2", bufs=2, space="PSUM") as ps_o2,
        ):
            for half in range(2):
                wfc_t = wmlp.tile([P, DT, HHT * P], R, tag="wfc")
                for dt_ in range(DT):
                    nc.gpsimd.dma_start(
                        wfc_t[:, dt_],
                        w_d["Wfc"].ap()[dt_ * P:(dt_ + 1) * P,
                                        half * HHT * P:(half + 1) * HHT * P])
                wpr_t = wmlp.tile([P, HHT, D], R, tag="wpr")
                for ht in range(HHT):
                    g = half * HHT + ht
                    nc.gpsimd.dma_start(wpr_t[:, ht], w_d["Wproj"].ap()[g * P:(g + 1) * P, :])

                for sc in range(NCH):
                    h2Tc = ph5.tile([P, DT, CH], R, tag="h2c")
                    nc.sync.dma_start(
                        h2Tc, h2_scr[:, :, sc * CH:(sc + 1) * CH].rearrange("t p n -> p t n"))

                    # uT + gelu -> mT
                    mT_sb = mtp.tile([P, HHT, CH], R, tag="mt")
                    for ht in range(HHT):
                        g = half * HHT + ht
                        ps = ps_u.tile([P, CH], F, tag="u")
                        for dt_ in range(DT):
                            nc.tensor.matmul(
                                ps,
                                wfc_t[:, dt_, ht * P:(ht + 1) * P],
                                h2Tc[:, dt_],
                                start=(dt_ == 0), stop=(dt_ == DT - 1))
                        nc.scalar.activation(out=mT_sb[:, ht], in_=ps, func=AF.Gelu,
                                             bias=bfc_col[:, g:g + 1], scale=1.0)

                    # o2 = mT.T @ Wproj
                    for su in range(TPC):
                        st = sc * TPC + su
                        o2_t = ph5.tile([P, D], F, tag="o2")
                        for dc in range(2):
                            ps = ps_o2.tile([P, D2C], F, tag="o2p")
                            for ht in range(HHT):
                                nc.tensor.matmul(
                                    ps,
                                    mT_sb[:, ht, su * P:(su + 1) * P],
                                    wpr_t[:, ht, dc * D2C:(dc + 1) * D2C],
                                    start=(ht == 0), stop=(ht == HHT - 1))
                            nc.vector.tensor_copy(out=o2_t[:, dc * D2C:(dc + 1) * D2C], in_=ps)
                        if half == 0:
                            nc.sync.dma_start(o2_scr[st], o2_t)
                        else:
                            prev = ph5.tile([P, D], F, tag="prev")
                            nc.sync.dma_start(prev, o2_scr[st])
                            x2_t = ph5.tile([P, D], F, tag="x2b")
                            nc.sync.dma_start(x2_t, x2_scr[st])
                            nc.vector.tensor_tensor(out=o2_t, in0=o2_t, in1=prev, op=OP.add)
                            nc.vector.tensor_tensor(out=o2_t, in0=o2_t, in1=x2_t, op=OP.add)
                            nc.vector.tensor_tensor(out=o2_t, in0=o2_t, in1=bp_bc, op=OP.add)
                            nc.sync.dma_start(out_d.ap()[st * P:(st + 1) * P, :], o2_t)

    return nc


def _get_nc():
    if "nc" not in _CACHE:
        nc = _build()
        nc.compile()
        _CACHE["nc"] = nc
    return _CACHE["nc"]


TRACE = False


def kernel(**inputs):
    from concourse.bass_utils import run_bass_kernel_spmd

    nc = _get_nc()
    x = np.asarray(inputs["x"], dtype=np.float32)
    base = {nm: np.ascontiguousarray(np.asarray(inputs[nm], dtype=np.float32))
            for nm in WEIGHT_NAMES}
    in_maps = [dict(base, x=np.ascontiguousarray(x[b])) for b in range(N_CORES)]
    res = run_bass_kernel_spmd(nc, in_maps, core_ids=list(range(N_CORES)), trace=TRACE)
    _CACHE["last_res"] = res
    return np.stack([res.results[b]["out"] for b in range(N_CORES)], axis=0)
